# revision 16
# baseline (speedup 1.0000x reference)
"""Fused multi-head attention for Trainium2, SPMD across 8 NeuronCores.

Problem: B=2, T=2048, D=1024, H=16 heads (DH=64), fp32.
Returns (out[B,T,D], att_w[B,H,T,T]) matching the reference nn.Module.

Sharding: core c -> (batch b = c//4, head-group g = c%4).  Each core owns
4 heads of one batch: Wq/Wk/Wv column slice [:, 256g:256g+256], Wo row
slice [256g:256g+256, :].  Output projection partials are summed on host
(tensor-parallel reduction); att_w shards concatenate.

Per-core pipeline (PE is pinned at 1.2 GHz here, so every [128,512] psum
tile costs ~512 PE cycles regardless of dtype/K):

  NATURAL side (per head, fp32r):  s[q,k] = q.k/8 + mask_k*(-1e9) in one
  K=65 matmul (ones row 64 in the Q operand, mask row 64 in the K
  operand).  ACT exp with accum_out yields unnormalized att tiles plus
  their row sums (softmax denominators) for free.  DVE tensor_scalar
  multiplies by 1/denom (per-partition) -> normalized att_w -> DMA out.

  TRANSPOSED side (per head-PAIR, bf16, row-packed):  two K=64 matmuls in
  disjoint PE row strips run concurrently (head 2p at partitions 0-63,
  2p+1 at 64-127 of bf16 pair tiles); the mask is applied as the exp's
  per-partition bias.  u[k,q] = exp(sT) feeds the AV matmul (fp32r),
  giving unnormalized out^T per head; normalization uses a PE rank-1
  broadcast of the natural-side 1/denom + a DVE multiply, then the
  output projection runs over the 4 heads.
"""

import sys

if "/opt/trn_rl_repo" not in sys.path:
    sys.path.insert(0, "/opt/trn_rl_repo")

import numpy as np

import concourse.bacc as bacc
import concourse.tile as tile
import concourse.mybir as mybir
from concourse.bass_utils import run_bass_kernel_spmd
from concourse.masks import make_identity

F32 = mybir.dt.float32
F32R = mybir.dt.float32r
BF16 = mybir.dt.bfloat16
AF = mybir.ActivationFunctionType
ALU = mybir.AluOpType
AX = mybir.AxisListType

B, T, D, H = 2, 2048, 1024, 16
DH = D // H
NCORES = 8
HPC = 4            # heads per core
FPC = HPC * DH     # 256 features per core
KT = D // 128      # 8 k-tiles for projections
TT = T // 128      # 16 token tiles
XC = 2             # x-stream chunk: k-tiles per DMA

TRACE = False
LAST_RESULTS = None


def build_nc():
    nc = bacc.Bacc("TRN2", target_bir_lowering=False, debug=False,
                   num_devices=NCORES)

    # ---- DRAM I/O (per-core shard, host-prepared layouts) ----
    xq = nc.dram_tensor("xq", [128, KT, T], F32, kind="ExternalInput")
    xk = nc.dram_tensor("xk", [128, KT, T], F32, kind="ExternalInput")
    xv = nc.dram_tensor("xv", [128, KT, T], F32, kind="ExternalInput")
    wq = nc.dram_tensor("wq", [128, KT, FPC], F32, kind="ExternalInput")  # /8
    wk = nc.dram_tensor("wk", [128, KT, FPC], F32, kind="ExternalInput")
    wv = nc.dram_tensor("wv", [128, KT, FPC], F32, kind="ExternalInput")
    wo = nc.dram_tensor("wo", [128, 2, D], F32, kind="ExternalInput")
    bq = nc.dram_tensor("bq", [128, 2], F32, kind="ExternalInput")        # /8
    bk = nc.dram_tensor("bk", [128, 2], F32, kind="ExternalInput")
    maskneg = nc.dram_tensor("maskneg", [1, T], F32, kind="ExternalInput")
    mbias = nc.dram_tensor("mbias", [128, TT], F32, kind="ExternalInput")
    onesT = nc.dram_tensor("onesT", [1, T], F32, kind="ExternalInput")
    ones64 = nc.dram_tensor("ones64", [1, DH], F32, kind="ExternalInput")
    identr = nc.dram_tensor("identr", [128, 128], F32, kind="ExternalInput")
    att4 = nc.dram_tensor("att4", [HPC, T, T], F32, kind="ExternalOutput")
    yp = nc.dram_tensor("yp", [T, D], F32, kind="ExternalOutput")

    with tile.TileContext(nc) as tc:
        # ---------- persistent SBUF ----------
        with tc.tile_pool(name="persist", bufs=1) as pp:
            # natural-side per-head tiles (f32r, aug row 64)
            qaug = [pp.tile([128, T], F32R, tag=f"qaug{h}", name=f"qaug{h}")
                    for h in range(HPC)]
            kaug = [pp.tile([128, T], F32R, tag=f"kaug{h}", name=f"kaug{h}")
                    for h in range(HPC)]
            # transposed-side pair tiles (bf16; head 2p rows 0-63,
            # 2p+1 rows 64-127)
            qpair = [pp.tile([128, T], BF16, tag=f"qpair{p}", name=f"qpair{p}")
                     for p in range(2)]
            kpair = [pp.tile([128, T], BF16, tag=f"kpair{p}", name=f"kpair{p}")
                     for p in range(2)]
            # V token-major per head (f32r): [tok128, kt, DH]
            vaug = [pp.tile([128, TT, DH], F32R, tag=f"vaug{h}",
                            name=f"vaug{h}") for h in range(HPC)]
            at01 = pp.tile([128, T], F32R, tag="at01")   # A^T heads 0,1
            at23 = pp.tile([128, T], F32R, tag="at23")   # A^T heads 2,3
            ident = pp.tile([128, 128], F32, tag="ident")
            identr_sb = pp.tile([128, 128], F32R, tag="identr_sb")
            ones_row = pp.tile([1, DH], F32R, tag="ones_row")
            mb = pp.tile([128, TT], F32, tag="mb")
            wo_sb = pp.tile([128, 2, D], F32R, tag="wo_sb")
            nc.gpsimd.dma_start(out=wo_sb[:], in_=wo[:])

            make_identity(nc, ident[:])
            nc.gpsimd.dma_start(out=identr_sb[:], in_=identr[:])
            nc.gpsimd.dma_start(out=ones_row[:], in_=ones64[:])
            nc.sync.dma_start(out=mb[:], in_=mbias[:])
            for h in range(HPC):
                nc.gpsimd.dma_start(out=qaug[h][64:65, :], in_=onesT[:])
                nc.gpsimd.dma_start(out=kaug[h][64:65, :], in_=maskneg[:])

            # ---------- phase 1: projections ----------
            with tc.tile_pool(name="p1w", bufs=1) as p1w, \
                 tc.tile_pool(name="p1x", bufs=2) as p1x, \
                 tc.tile_pool(name="p1v", bufs=4) as p1v, \
                 tc.tile_pool(name="p1ps", bufs=1, space="PSUM") as p1ps:
                w_sb = {}
                for nm, wt in (("q", wq), ("k", wk), ("v", wv)):
                    w_sb[nm] = p1w.tile([128, KT, FPC], F32R, tag=f"w{nm}",
                                        name=f"w{nm}")
                    nc.gpsimd.dma_start(out=w_sb[nm][:], in_=wt[:])
                b_sb = {}
                for nm, bt in (("q", bq), ("k", bk)):
                    b_sb[nm] = p1w.tile([128, 2], F32, tag=f"b{nm}",
                                        name=f"b{nm}")
                    nc.sync.dma_start(out=b_sb[nm][:], in_=bt[:])

                def project(xdram, wname, evac):
                    """PT[f,t] = sum_K W[K,f] X^T[K,t]; evac(ps, m, n)."""
                    ps = [p1ps.tile([128, 512], F32, tag=f"ps{m}_{n}",
                                    name=f"ps{m}_{n}")
                          for m in range(2) for n in range(4)]
                    for kc in range(KT // XC):
                        xt = p1x.tile([128, XC, T], F32R, tag="xstream",
                                      name="xt")
                        nc.gpsimd.dma_start(
                            out=xt[:], in_=xdram[:, kc * XC:(kc + 1) * XC, :])
                        for kj in range(XC):
                            kt = kc * XC + kj
                            for m in range(2):
                                for n in range(4):
                                    nc.tensor.matmul(
                                        ps[m * 4 + n][:],
                                        w_sb[wname][:, kt,
                                                    m * 128:(m + 1) * 128],
                                        xt[:, kj, n * 512:(n + 1) * 512],
                                        start=(kt == 0), stop=(kt == KT - 1))
                    for m in range(2):
                        for n in range(4):
                            evac(ps[m * 4 + n], m, n)

                def evac_qk(dst, pair, bias):
                    def evac(ps, m, n):
                        cols = slice(n * 512, (n + 1) * 512)
                        # per-head f32r rows 0-63 (natural side)
                        for hh in range(2):
                            h = 2 * m + hh
                            nc.vector.tensor_scalar(
                                out=dst[h][0:DH, cols],
                                in0=ps[hh * DH:(hh + 1) * DH, :],
                                scalar1=bias[hh * DH:(hh + 1) * DH, m:m + 1],
                                scalar2=None, op0=ALU.add)
                        # pair bf16 copy (transposed side), no partition shift
                        nc.scalar.activation(
                            pair[m][:, cols], ps[:], AF.Identity,
                            bias=bias[:, m:m + 1])
                    return evac

                project(xq, "q", evac_qk(qaug, qpair, b_sb["q"]))
                project(xk, "k", evac_qk(kaug, kpair, b_sb["k"]))

                # V: psum -> block staging -> PE transpose -> vaug
                def evac_v(ps, m, n):
                    for j in range(4):
                        tb = n * 4 + j
                        vb = p1v.tile([128, 128], F32, tag="vblk", name="vblk")
                        nc.vector.tensor_copy(vb[:],
                                              ps[:, j * 128:(j + 1) * 128])
                        pt = p1ps.tile([128, 128], F32, tag=f"ps{m}_{n}",
                                       name="ptr")
                        nc.tensor.transpose(pt[:], vb[:], ident[:])
                        for hh in range(2):
                            h = 2 * m + hh
                            nc.vector.tensor_copy(
                                vaug[h][:, tb, :],
                                pt[:, hh * DH:(hh + 1) * DH])

                project(xv, "v", evac_v)

            # ---------- phase 2: attention ----------
            with tc.tile_pool(name="p2u", bufs=3) as p2u, \
                 tc.tile_pool(name="p2att", bufs=3) as p2att, \
                 tc.tile_pool(name="p2sm", bufs=3) as p2sm, \
                 tc.tile_pool(name="psAB", bufs=2, space="PSUM") as psAB_p, \
                 tc.tile_pool(name="psV", bufs=2, space="PSUM") as psV_p, \
                 tc.tile_pool(name="psN", bufs=2, space="PSUM") as psN_p:
                for q2 in range(4):      # 512-wide query blocks
                    for p in range(2):
                        hA, hB = 2 * p, 2 * p + 1
                        q0 = q2 * 512
                        qc = slice(q0, q0 + 512)
                        # ---- transposed side: row-packed bf16 pair ----
                        pavA = psV_p.tile([DH, 512], F32, tag="pav",
                                          name="pavA")
                        pavB = psV_p.tile([DH, 512], F32, tag="pav",
                                          name="pavB")
                        for kt in range(TT):
                            kc = slice(kt * 128, (kt + 1) * 128)
                            psab = psAB_p.tile([128, 1024], F32, tag="psab",
                                               name="psab")
                            nc.tensor.matmul(psab[:, 0:512],
                                             kpair[p][0:DH, kc],
                                             qpair[p][0:DH, qc],
                                             start=True, stop=True)
                            nc.tensor.matmul(psab[:, 512:1024],
                                             kpair[p][DH:128, kc],
                                             qpair[p][DH:128, qc],
                                             start=True, stop=True,
                                             tile_position=(64, 0))
                            uA = p2u.tile([128, 512], F32R, tag="uA",
                                          name="uA")
                            nc.scalar.activation(uA[:], psab[:, 0:512], AF.Exp,
                                                 bias=mb[:, kt:kt + 1])
                            uB = p2u.tile([128, 512], F32R, tag="uB",
                                          name="uB")
                            nc.scalar.activation(uB[:], psab[:, 512:1024],
                                                 AF.Exp,
                                                 bias=mb[:, kt:kt + 1])
                            nc.tensor.matmul(pavA[:], vaug[hA][:, kt, :],
                                             uA[:], start=(kt == 0),
                                             stop=(kt == TT - 1))
                            nc.tensor.matmul(pavB[:], vaug[hB][:, kt, :],
                                             uB[:], start=(kt == 0),
                                             stop=(kt == TT - 1))

                        # ---- natural side + denominators, heads A and B ----
                        rrow = {}
                        for h in (hA, hB):
                            rr = p2sm.tile([1, 512], F32R, tag="rrow",
                                           name=f"rr{h}")
                            for qs in range(4):
                                row0 = q0 + qs * 128
                                asb = p2att.tile([128, T], F32, tag="asb",
                                                 name="asb")
                                accs = p2sm.tile([128, 4], F32, tag="accs",
                                                 name="accs")
                                for kn in range(4):
                                    psn = psN_p.tile([128, 512], F32,
                                                     tag="psn", name="psn")
                                    nc.tensor.matmul(
                                        psn[:],
                                        qaug[h][0:DH + 1, row0:row0 + 128],
                                        kaug[h][0:DH + 1,
                                                kn * 512:(kn + 1) * 512],
                                        start=True, stop=True)
                                    nc.scalar.activation(
                                        asb[:, kn * 512:(kn + 1) * 512],
                                        psn[:], AF.Exp,
                                        accum_out=accs[:, kn:kn + 1])
                                # denom = sum of chunk accums; reciprocal
                                dcol = p2sm.tile([128, 1], F32, tag="dcol",
                                                 name="dcol")
                                nc.vector.reduce_sum(dcol[:], accs[:],
                                                     axis=AX.X)
                                rcol = p2sm.tile([128, 1], F32, tag="rcol",
                                                 name="rcol")
                                nc.vector.reciprocal(rcol[:], dcol[:])
                                # normalize att rows in place, then DMA out
                                nc.vector.tensor_scalar(
                                    out=asb[:], in0=asb[:], scalar1=rcol[:],
                                    scalar2=None, op0=ALU.mult)
                                nc.sync.dma_start(
                                    out=att4[h, row0:row0 + 128, :],
                                    in_=asb[:])
                                # recip row segment for outT normalization
                                rcolr = p2sm.tile([128, 1], F32R, tag="rcolr",
                                                  name="rcolr")
                                nc.vector.tensor_copy(rcolr[:], rcol[:])
                                prow = psN_p.tile([1, 128], F32, tag="psn",
                                                  name="prow")
                                nc.tensor.matmul(prow[:], rcolr[:],
                                                 identr_sb[:],
                                                 start=True, stop=True)
                                nc.vector.tensor_copy(
                                    rr[0:1, qs * 128:(qs + 1) * 128],
                                    prow[:])
                            rrow[h] = rr

                        # ---- outT normalize into AT tiles ----
                        at_dst = at01 if p == 0 else at23
                        for h, pav in ((hA, pavA), (hB, pavB)):
                            prc = psN_p.tile([DH, 512], F32, tag="psn",
                                             name="prc")
                            nc.tensor.matmul(prc[:], ones_row[:],
                                             rrow[h][:], start=True,
                                             stop=True)
                            rbc = p2sm.tile([DH, 512], F32, tag="rbc",
                                            name="rbc")
                            nc.vector.tensor_copy(rbc[:], prc[:])
                            nc.vector.tensor_mul(
                                at_dst[(h % 2) * DH:(h % 2) * DH + DH, qc],
                                pav[:], rbc[:])

                    # ---- output projection for this q2 block ----
                    for tj in range(4):
                        ts_ = q2 * 4 + tj
                        ysb = p2att.tile([128, D], F32, tag="ysb", name="ysb")
                        for n in range(2):
                            py = psAB_p.tile([128, 512], F32, tag="psab",
                                             name="py")
                            nc.tensor.matmul(
                                py[:], at01[:, ts_ * 128:(ts_ + 1) * 128],
                                wo_sb[:, 0, n * 512:(n + 1) * 512],
                                start=True, stop=False)
                            nc.tensor.matmul(
                                py[:], at23[:, ts_ * 128:(ts_ + 1) * 128],
                                wo_sb[:, 1, n * 512:(n + 1) * 512],
                                start=False, stop=True)
                            nc.scalar.copy(ysb[:, n * 512:(n + 1) * 512],
                                           py[:])
                        nc.gpsimd.dma_start(
                            out=yp[ts_ * 128:(ts_ + 1) * 128, :], in_=ysb[:])

    nc.compile()
    return nc


_NC = None


def kernel(query=None, key_in=None, value=None, mask=None, Wq=None, bq=None,
           Wk=None, bk=None, Wv=None, bv=None, Wo=None, bo=None, key=None,
           **_ignored):
    global _NC, LAST_RESULTS
    if key_in is None:
        key_in = key
    query = np.ascontiguousarray(np.asarray(query, dtype=np.float32))
    key_in = np.ascontiguousarray(np.asarray(key_in, dtype=np.float32))
    value = np.ascontiguousarray(np.asarray(value, dtype=np.float32))
    mask_np = np.asarray(mask)
    Wq = np.asarray(Wq, dtype=np.float32)
    Wk = np.asarray(Wk, dtype=np.float32)
    Wv = np.asarray(Wv, dtype=np.float32)
    Wo = np.asarray(Wo, dtype=np.float32)
    bq_np = np.asarray(bq, dtype=np.float32)
    bk_np = np.asarray(bk, dtype=np.float32)
    bv_np = np.asarray(bv, dtype=np.float32)
    bo_np = np.asarray(bo, dtype=np.float32)

    if _NC is None:
        _NC = build_nc()

    scale = 1.0 / np.sqrt(np.float32(DH))
    xT = {}
    for b in range(B):
        # [128, KT, T]: partition-major X^T
        xT[("q", b)] = np.ascontiguousarray(
            query[b].T.reshape(KT, 128, T).transpose(1, 0, 2))
        xT[("k", b)] = np.ascontiguousarray(
            key_in[b].T.reshape(KT, 128, T).transpose(1, 0, 2))
        xT[("v", b)] = np.ascontiguousarray(
            value[b].T.reshape(KT, 128, T).transpose(1, 0, 2))
    maskneg = {b: (mask_np[b, 0, 0, :].astype(np.float32) * np.float32(-1e9))
               for b in range(B)}

    in_maps = []
    for c in range(NCORES):
        b, g = c // HPC, c % HPC
        F = slice(g * FPC, (g + 1) * FPC)
        in_maps.append({
            "xq": xT[("q", b)],
            "xk": xT[("k", b)],
            "xv": xT[("v", b)],
            "wq": np.ascontiguousarray(
                (Wq[:, F] * scale).reshape(KT, 128, FPC).transpose(1, 0, 2)),
            "wk": np.ascontiguousarray(
                Wk[:, F].reshape(KT, 128, FPC).transpose(1, 0, 2)),
            "wv": np.ascontiguousarray(
                Wv[:, F].reshape(KT, 128, FPC).transpose(1, 0, 2)),
            "wo": np.ascontiguousarray(
                Wo[F, :].reshape(2, 128, D).transpose(1, 0, 2)),
            "bq": np.ascontiguousarray((bq_np[F] * scale).reshape(2, 128).T),
            "bk": np.ascontiguousarray(bk_np[F].reshape(2, 128).T),
            "maskneg": maskneg[b].reshape(1, T),
            "mbias": np.ascontiguousarray(maskneg[b].reshape(TT, 128).T),
            "onesT": np.ones((1, T), np.float32),
            "ones64": np.ones((1, DH), np.float32),
            "identr": np.eye(128, dtype=np.float32),
        })

    res = run_bass_kernel_spmd(_NC, in_maps, list(range(NCORES)), trace=TRACE)
    LAST_RESULTS = res

    att_w = np.empty((B, H, T, T), dtype=np.float32)
    out = np.zeros((B, T, D), dtype=np.float32)
    for c in range(NCORES):
        b, g = c // HPC, c % HPC
        att_w[b, g * HPC:(g + 1) * HPC] = res.results[c]["att4"]
        out[b] += res.results[c]["yp"]
    out += (bv_np @ Wo + bo_np)[None, None, :]
    return out, att_w


# revision 18
# speedup vs baseline: 1.0952x; 1.0952x over previous
"""Fused multi-head attention for Trainium2, SPMD across 8 NeuronCores.

Problem: B=2, T=2048, D=1024, H=16 heads (DH=64), fp32.
Returns (out[B,T,D], att_w[B,H,T,T]) matching the reference nn.Module.

Sharding: core c -> (batch b = c//4, head-group g = c%4).  Each core owns
4 heads of one batch: Wq/Wk/Wv column slice [:, 256g:256g+256], Wo row
slice [256g:256g+256, :].  Output projection partials are summed on host
(tensor-parallel reduction); att_w shards concatenate.

Per-core pipeline (PE is pinned at 1.2 GHz here, so every [128,512] psum
tile costs ~512 PE cycles regardless of dtype/K):

  NATURAL side (per head, fp32r):  s[q,k] = q.k/8 + mask_k*(-1e9) in one
  K=65 matmul (ones row 64 in the Q operand, mask row 64 in the K
  operand).  ACT exp with accum_out yields unnormalized att tiles plus
  their row sums (softmax denominators) for free.  DVE tensor_scalar
  multiplies by 1/denom (per-partition) -> normalized att_w -> DMA out.

  TRANSPOSED side (per head-PAIR, bf16, row-packed):  two K=64 matmuls in
  disjoint PE row strips run concurrently (head 2p at partitions 0-63,
  2p+1 at 64-127 of bf16 pair tiles); the mask is applied as the exp's
  per-partition bias.  u[k,q] = exp(sT) feeds the AV matmul (fp32r),
  giving unnormalized out^T per head; normalization uses a PE rank-1
  broadcast of the natural-side 1/denom + a DVE multiply, then the
  output projection runs over the 4 heads.
"""

import sys

if "/opt/trn_rl_repo" not in sys.path:
    sys.path.insert(0, "/opt/trn_rl_repo")

import numpy as np

import concourse.bacc as bacc
import concourse.tile as tile
import concourse.mybir as mybir
from concourse.bass_utils import run_bass_kernel_spmd
from concourse.masks import make_identity

F32 = mybir.dt.float32
F32R = mybir.dt.float32r
BF16 = mybir.dt.bfloat16
AF = mybir.ActivationFunctionType
ALU = mybir.AluOpType
AX = mybir.AxisListType

B, T, D, H = 2, 2048, 1024, 16
DH = D // H
NCORES = 8
HPC = 4            # heads per core
FPC = HPC * DH     # 256 features per core
KT = D // 128      # 8 k-tiles for projections
TT = T // 128      # 16 token tiles
XC = 2             # x-stream chunk: k-tiles per DMA

TRACE = False
LAST_RESULTS = None


def build_nc():
    nc = bacc.Bacc("TRN2", target_bir_lowering=False, debug=False,
                   num_devices=NCORES)

    # ---- DRAM I/O (per-core shard, host-prepared layouts) ----
    xq = nc.dram_tensor("xq", [128, KT, T], F32, kind="ExternalInput")
    xk = nc.dram_tensor("xk", [128, KT, T], F32, kind="ExternalInput")
    xv = nc.dram_tensor("xv", [128, KT, T], F32, kind="ExternalInput")
    wq = nc.dram_tensor("wq", [128, KT, FPC], F32, kind="ExternalInput")  # /8
    wk = nc.dram_tensor("wk", [128, KT, FPC], F32, kind="ExternalInput")
    wv = nc.dram_tensor("wv", [128, KT, FPC], F32, kind="ExternalInput")
    wo = nc.dram_tensor("wo", [128, 2, D], F32, kind="ExternalInput")
    bq = nc.dram_tensor("bq", [128, 2], F32, kind="ExternalInput")        # /8
    bk = nc.dram_tensor("bk", [128, 2], F32, kind="ExternalInput")
    maskneg = nc.dram_tensor("maskneg", [1, T], F32, kind="ExternalInput")
    mbias = nc.dram_tensor("mbias", [128, TT], F32, kind="ExternalInput")
    onesT = nc.dram_tensor("onesT", [1, T], F32, kind="ExternalInput")
    ones64 = nc.dram_tensor("ones64", [1, DH], F32, kind="ExternalInput")
    onesV = nc.dram_tensor("onesV", [128, TT], F32, kind="ExternalInput")
    att4 = nc.dram_tensor("att4", [HPC, T, T], F32, kind="ExternalOutput")
    yp = nc.dram_tensor("yp", [T, D], F32, kind="ExternalOutput")

    with tile.TileContext(nc) as tc:
        # ---------- persistent SBUF ----------
        with tc.tile_pool(name="persist", bufs=1) as pp:
            # natural-side per-head tiles (f32r, aug row 64)
            qaug = [pp.tile([128, T], F32R, tag=f"qaug{h}", name=f"qaug{h}")
                    for h in range(HPC)]
            kaug = [pp.tile([128, T], F32R, tag=f"kaug{h}", name=f"kaug{h}")
                    for h in range(HPC)]
            # transposed-side pair tiles (bf16; head 2p rows 0-63,
            # 2p+1 rows 64-127)
            qpair = [pp.tile([128, T], BF16, tag=f"qpair{p}", name=f"qpair{p}")
                     for p in range(2)]
            kpair = [pp.tile([128, T], BF16, tag=f"kpair{p}", name=f"kpair{p}")
                     for p in range(2)]
            # V token-major per head (f32r): [tok128, kt, DH + ones col]
            vaug = [pp.tile([128, TT, DH + 1], F32R, tag=f"vaug{h}",
                            name=f"vaug{h}") for h in range(HPC)]
            at01 = pp.tile([128, T], F32R, tag="at01")   # A^T heads 0,1
            at23 = pp.tile([128, T], F32R, tag="at23")   # A^T heads 2,3
            ident = pp.tile([128, 128], F32, tag="ident")
            ones_row = pp.tile([1, DH], F32R, tag="ones_row")
            mb = pp.tile([128, TT], F32, tag="mb")
            wo_sb = pp.tile([128, 2, D], F32R, tag="wo_sb")
            nc.gpsimd.dma_start(out=wo_sb[:], in_=wo[:])

            make_identity(nc, ident[:])
            nc.gpsimd.dma_start(out=ones_row[:], in_=ones64[:])
            nc.sync.dma_start(out=mb[:], in_=mbias[:])
            for h in range(HPC):
                nc.gpsimd.dma_start(out=qaug[h][64:65, :], in_=onesT[:])
                nc.gpsimd.dma_start(out=kaug[h][64:65, :], in_=maskneg[:])
                nc.gpsimd.dma_start(out=vaug[h][:, :, DH:DH + 1],
                                    in_=onesV[:, :, None])

            # ---------- phase 1: projections ----------
            with tc.tile_pool(name="p1w", bufs=1) as p1w, \
                 tc.tile_pool(name="p1x", bufs=2) as p1x, \
                 tc.tile_pool(name="p1v", bufs=4) as p1v, \
                 tc.tile_pool(name="p1ps", bufs=1, space="PSUM") as p1ps:
                w_sb = {}
                for nm, wt in (("q", wq), ("k", wk), ("v", wv)):
                    w_sb[nm] = p1w.tile([128, KT, FPC], F32R, tag=f"w{nm}",
                                        name=f"w{nm}")
                    nc.gpsimd.dma_start(out=w_sb[nm][:], in_=wt[:])
                b_sb = {}
                for nm, bt in (("q", bq), ("k", bk)):
                    b_sb[nm] = p1w.tile([128, 2], F32, tag=f"b{nm}",
                                        name=f"b{nm}")
                    nc.sync.dma_start(out=b_sb[nm][:], in_=bt[:])

                def project(xdram, wname, evac):
                    """PT[f,t] = sum_K W[K,f] X^T[K,t]; evac(ps, m, n)."""
                    ps = [p1ps.tile([128, 512], F32, tag=f"ps{m}_{n}",
                                    name=f"ps{m}_{n}")
                          for m in range(2) for n in range(4)]
                    for kc in range(KT // XC):
                        xt = p1x.tile([128, XC, T], F32R, tag="xstream",
                                      name="xt")
                        nc.gpsimd.dma_start(
                            out=xt[:], in_=xdram[:, kc * XC:(kc + 1) * XC, :])
                        for kj in range(XC):
                            kt = kc * XC + kj
                            for m in range(2):
                                for n in range(4):
                                    nc.tensor.matmul(
                                        ps[m * 4 + n][:],
                                        w_sb[wname][:, kt,
                                                    m * 128:(m + 1) * 128],
                                        xt[:, kj, n * 512:(n + 1) * 512],
                                        start=(kt == 0), stop=(kt == KT - 1))
                    for m in range(2):
                        for n in range(4):
                            evac(ps[m * 4 + n], m, n)

                def evac_qk(dst, pair, bias):
                    def evac(ps, m, n):
                        cols = slice(n * 512, (n + 1) * 512)
                        # per-head f32r rows 0-63 (natural side)
                        for hh in range(2):
                            h = 2 * m + hh
                            nc.vector.tensor_scalar(
                                out=dst[h][0:DH, cols],
                                in0=ps[hh * DH:(hh + 1) * DH, :],
                                scalar1=bias[hh * DH:(hh + 1) * DH, m:m + 1],
                                scalar2=None, op0=ALU.add)
                        # pair bf16 copy (transposed side), no partition shift
                        nc.vector.tensor_scalar(
                            out=pair[m][:, cols], in0=ps[:],
                            scalar1=bias[:, m:m + 1], scalar2=None,
                            op0=ALU.add)
                    return evac

                project(xq, "q", evac_qk(qaug, qpair, b_sb["q"]))
                project(xk, "k", evac_qk(kaug, kpair, b_sb["k"]))

                # V: psum -> block staging -> PE transpose -> vaug
                def evac_v(ps, m, n):
                    for j in range(4):
                        tb = n * 4 + j
                        vb = p1v.tile([128, 128], F32, tag="vblk", name="vblk")
                        nc.vector.tensor_copy(vb[:],
                                              ps[:, j * 128:(j + 1) * 128])
                        pt = p1ps.tile([128, 128], F32, tag=f"ps{m}_{n}",
                                       name="ptr")
                        nc.tensor.transpose(pt[:], vb[:], ident[:])
                        for hh in range(2):
                            h = 2 * m + hh
                            nc.vector.tensor_copy(
                                vaug[h][:, tb, 0:DH],
                                pt[:, hh * DH:(hh + 1) * DH])

                project(xv, "v", evac_v)

            # ---------- phase 2: attention ----------
            with tc.tile_pool(name="p2u", bufs=3) as p2u, \
                 tc.tile_pool(name="p2att", bufs=3) as p2att, \
                 tc.tile_pool(name="p2sm", bufs=3) as p2sm, \
                 tc.tile_pool(name="psAB", bufs=2, space="PSUM") as psAB_p, \
                 tc.tile_pool(name="psV", bufs=2, space="PSUM") as psV_p, \
                 tc.tile_pool(name="psN", bufs=1, space="PSUM") as psN_p:
                for q2 in range(4):      # 512-wide query blocks
                    for p in range(2):
                        hA, hB = 2 * p, 2 * p + 1
                        q0 = q2 * 512
                        qc = slice(q0, q0 + 512)
                        # ---- transposed side: row-packed bf16 pair ----
                        pavA = psV_p.tile([DH + 1, 512], F32, tag="pav",
                                          name="pavA")
                        pavB = psV_p.tile([DH + 1, 512], F32, tag="pav",
                                          name="pavB")
                        for kt in range(TT):
                            kc = slice(kt * 128, (kt + 1) * 128)
                            psab = psAB_p.tile([128, 1024], F32, tag="psab",
                                               name="psab")
                            nc.tensor.matmul(psab[:, 0:512],
                                             kpair[p][0:DH, kc],
                                             qpair[p][0:DH, qc],
                                             start=True, stop=True)
                            nc.tensor.matmul(psab[:, 512:1024],
                                             kpair[p][DH:128, kc],
                                             qpair[p][DH:128, qc],
                                             start=True, stop=True,
                                             tile_position=(64, 0))
                            uA = p2u.tile([128, 512], F32R, tag="uA",
                                          name="uA")
                            nc.scalar.activation(uA[:], psab[:, 0:512], AF.Exp,
                                                 bias=mb[:, kt:kt + 1])
                            uB = p2u.tile([128, 512], F32R, tag="uB",
                                          name="uB")
                            nc.scalar.activation(uB[:], psab[:, 512:1024],
                                                 AF.Exp,
                                                 bias=mb[:, kt:kt + 1])
                            nc.tensor.matmul(pavA[:], vaug[hA][:, kt, :],
                                             uA[:], start=(kt == 0),
                                             stop=(kt == TT - 1))
                            nc.tensor.matmul(pavB[:], vaug[hB][:, kt, :],
                                             uB[:], start=(kt == 0),
                                             stop=(kt == TT - 1))

                        # ---- natural side + denominators, heads A and B ----
                        for h in (hA, hB):
                            for qs in range(4):
                                row0 = q0 + qs * 128
                                asb = p2att.tile([128, T], F32, tag="asb",
                                                 name="asb")
                                accs = p2sm.tile([128, 2], F32, tag="accs",
                                                 name="accs")
                                for kn in range(2):
                                    psn = psN_p.tile([128, 1024], F32,
                                                     tag="psn", name="psn")
                                    for jj in range(2):
                                        nc.tensor.matmul(
                                            psn[:, jj * 512:(jj + 1) * 512],
                                            qaug[h][0:DH + 1, row0:row0 + 128],
                                            kaug[h][0:DH + 1,
                                                    (2 * kn + jj) * 512:
                                                    (2 * kn + jj + 1) * 512],
                                            start=True, stop=True)
                                    nc.scalar.activation(
                                        asb[:, kn * 1024:(kn + 1) * 1024],
                                        psn[:], AF.Exp,
                                        accum_out=accs[:, kn:kn + 1])
                                # denom = sum of chunk accums; reciprocal
                                dcol = p2sm.tile([128, 1], F32, tag="dcol",
                                                 name="dcol")
                                nc.vector.reduce_sum(dcol[:], accs[:],
                                                     axis=AX.X)
                                rcol = p2sm.tile([128, 1], F32, tag="rcol",
                                                 name="rcol")
                                nc.vector.reciprocal(rcol[:], dcol[:])
                                # normalize att rows in place, then DMA out
                                nc.vector.tensor_scalar(
                                    out=asb[:], in0=asb[:], scalar1=rcol[:],
                                    scalar2=None, op0=ALU.mult)
                                nc.sync.dma_start(
                                    out=att4[h, row0:row0 + 128, :],
                                    in_=asb[:])

                        # ---- outT normalize into AT tiles (denoms = AV row
                        # 64; 1/d = exp(-ln d) on ACT) ----
                        at_dst = at01 if p == 0 else at23
                        for h, pav in ((hA, pavA), (hB, pavB)):
                            lnr = p2sm.tile([1, 512], F32, tag="lnr",
                                            name="lnr")
                            nc.scalar.activation(lnr[:], pav[DH:DH + 1, :],
                                                 AF.Ln)
                            rrw = p2sm.tile([1, 512], F32R, tag="rrw",
                                            name="rrw")
                            nc.scalar.activation(rrw[:], lnr[:], AF.Exp,
                                                 scale=-1.0)
                            prc = psN_p.tile([DH, 512], F32, tag="psn",
                                             name="prc")
                            nc.tensor.matmul(prc[:], ones_row[:],
                                             rrw[:], start=True,
                                             stop=True)
                            rbc = p2sm.tile([DH, 512], F32, tag="rbc",
                                            name="rbc")
                            nc.vector.tensor_copy(rbc[:], prc[:])
                            nc.vector.tensor_mul(
                                at_dst[(h % 2) * DH:(h % 2) * DH + DH, qc],
                                pav[0:DH, :], rbc[:])

                    # ---- output projection for this q2 block ----
                    for tj in range(4):
                        ts_ = q2 * 4 + tj
                        ysb = p2att.tile([128, D], F32, tag="ysb", name="ysb")
                        for n in range(2):
                            py = psAB_p.tile([128, 512], F32, tag="psab",
                                             name="py")
                            nc.tensor.matmul(
                                py[:], at01[:, ts_ * 128:(ts_ + 1) * 128],
                                wo_sb[:, 0, n * 512:(n + 1) * 512],
                                start=True, stop=False)
                            nc.tensor.matmul(
                                py[:], at23[:, ts_ * 128:(ts_ + 1) * 128],
                                wo_sb[:, 1, n * 512:(n + 1) * 512],
                                start=False, stop=True)
                            nc.vector.tensor_copy(
                                ysb[:, n * 512:(n + 1) * 512], py[:])
                        nc.gpsimd.dma_start(
                            out=yp[ts_ * 128:(ts_ + 1) * 128, :], in_=ysb[:])

    nc.compile()
    return nc


_NC = None


def kernel(query=None, key_in=None, value=None, mask=None, Wq=None, bq=None,
           Wk=None, bk=None, Wv=None, bv=None, Wo=None, bo=None, key=None,
           **_ignored):
    global _NC, LAST_RESULTS
    if key_in is None:
        key_in = key
    query = np.ascontiguousarray(np.asarray(query, dtype=np.float32))
    key_in = np.ascontiguousarray(np.asarray(key_in, dtype=np.float32))
    value = np.ascontiguousarray(np.asarray(value, dtype=np.float32))
    mask_np = np.asarray(mask)
    Wq = np.asarray(Wq, dtype=np.float32)
    Wk = np.asarray(Wk, dtype=np.float32)
    Wv = np.asarray(Wv, dtype=np.float32)
    Wo = np.asarray(Wo, dtype=np.float32)
    bq_np = np.asarray(bq, dtype=np.float32)
    bk_np = np.asarray(bk, dtype=np.float32)
    bv_np = np.asarray(bv, dtype=np.float32)
    bo_np = np.asarray(bo, dtype=np.float32)

    if _NC is None:
        _NC = build_nc()

    scale = 1.0 / np.sqrt(np.float32(DH))
    xT = {}
    for b in range(B):
        # [128, KT, T]: partition-major X^T
        xT[("q", b)] = np.ascontiguousarray(
            query[b].T.reshape(KT, 128, T).transpose(1, 0, 2))
        xT[("k", b)] = np.ascontiguousarray(
            key_in[b].T.reshape(KT, 128, T).transpose(1, 0, 2))
        xT[("v", b)] = np.ascontiguousarray(
            value[b].T.reshape(KT, 128, T).transpose(1, 0, 2))
    maskneg = {b: (mask_np[b, 0, 0, :].astype(np.float32) * np.float32(-1e9))
               for b in range(B)}

    in_maps = []
    for c in range(NCORES):
        b, g = c // HPC, c % HPC
        F = slice(g * FPC, (g + 1) * FPC)
        in_maps.append({
            "xq": xT[("q", b)],
            "xk": xT[("k", b)],
            "xv": xT[("v", b)],
            "wq": np.ascontiguousarray(
                (Wq[:, F] * scale).reshape(KT, 128, FPC).transpose(1, 0, 2)),
            "wk": np.ascontiguousarray(
                Wk[:, F].reshape(KT, 128, FPC).transpose(1, 0, 2)),
            "wv": np.ascontiguousarray(
                Wv[:, F].reshape(KT, 128, FPC).transpose(1, 0, 2)),
            "wo": np.ascontiguousarray(
                Wo[F, :].reshape(2, 128, D).transpose(1, 0, 2)),
            "bq": np.ascontiguousarray((bq_np[F] * scale).reshape(2, 128).T),
            "bk": np.ascontiguousarray(bk_np[F].reshape(2, 128).T),
            "maskneg": maskneg[b].reshape(1, T),
            "mbias": np.ascontiguousarray(maskneg[b].reshape(TT, 128).T),
            "onesT": np.ones((1, T), np.float32),
            "ones64": np.ones((1, DH), np.float32),
            "onesV": np.ones((128, TT), np.float32),
        })

    res = run_bass_kernel_spmd(_NC, in_maps, list(range(NCORES)), trace=TRACE)
    LAST_RESULTS = res

    att_w = np.empty((B, H, T, T), dtype=np.float32)
    out = np.zeros((B, T, D), dtype=np.float32)
    for c in range(NCORES):
        b, g = c // HPC, c % HPC
        att_w[b, g * HPC:(g + 1) * HPC] = res.results[c]["att4"]
        out[b] += res.results[c]["yp"]
    out += (bv_np @ Wo + bo_np)[None, None, :]
    return out, att_w


# revision 20
# speedup vs baseline: 1.1545x; 1.0541x over previous
"""Fused multi-head attention for Trainium2, SPMD across 8 NeuronCores.

Problem: B=2, T=2048, D=1024, H=16 heads (DH=64), fp32.
Returns (out[B,T,D], att_w[B,H,T,T]) matching the reference nn.Module.

Sharding: core c -> (batch b = c//4, head-group g = c%4).  Each core owns
4 heads of one batch: Wq/Wk/Wv column slice [:, 256g:256g+256], Wo row
slice [256g:256g+256, :].  Output projection partials are summed on host
(tensor-parallel reduction); att_w shards concatenate.

Per-core pipeline (PE is pinned at 1.2 GHz here, so every [128,512] psum
tile costs ~512 PE cycles regardless of dtype/K):

  NATURAL side (per head, fp32r):  s[q,k] = q.k/8 + mask_k*(-1e9) in one
  K=65 matmul (ones row 64 in the Q operand, mask row 64 in the K
  operand).  ACT exp with accum_out yields unnormalized att tiles plus
  their row sums (softmax denominators) for free.  DVE tensor_scalar
  multiplies by 1/denom (per-partition) -> normalized att_w -> DMA out.

  TRANSPOSED side (per head-PAIR, bf16, row-packed):  two K=64 matmuls in
  disjoint PE row strips run concurrently (head 2p at partitions 0-63,
  2p+1 at 64-127 of bf16 pair tiles); the mask is applied as the exp's
  per-partition bias.  u[k,q] = exp(sT) feeds the AV matmul (fp32r),
  giving unnormalized out^T per head; normalization uses a PE rank-1
  broadcast of the natural-side 1/denom + a DVE multiply, then the
  output projection runs over the 4 heads.
"""

import sys

if "/opt/trn_rl_repo" not in sys.path:
    sys.path.insert(0, "/opt/trn_rl_repo")

import numpy as np

import concourse.bacc as bacc
import concourse.tile as tile
import concourse.mybir as mybir
from concourse.bass_utils import run_bass_kernel_spmd
from concourse.masks import make_identity

F32 = mybir.dt.float32
F32R = mybir.dt.float32r
BF16 = mybir.dt.bfloat16
AF = mybir.ActivationFunctionType
ALU = mybir.AluOpType
AX = mybir.AxisListType

B, T, D, H = 2, 2048, 1024, 16
DH = D // H
NCORES = 8
HPC = 4            # heads per core
FPC = HPC * DH     # 256 features per core
KT = D // 128      # 8 k-tiles for projections
TT = T // 128      # 16 token tiles
XC = 2             # x-stream chunk: k-tiles per DMA

TRACE = False
LAST_RESULTS = None


def build_nc():
    nc = bacc.Bacc("TRN2", target_bir_lowering=False, debug=False,
                   num_devices=NCORES)

    # ---- DRAM I/O (per-core shard, host-prepared layouts) ----
    xq = nc.dram_tensor("xq", [128, KT, T], F32, kind="ExternalInput")
    xk = nc.dram_tensor("xk", [128, KT, T], F32, kind="ExternalInput")
    xv = nc.dram_tensor("xv", [128, KT, T], F32, kind="ExternalInput")
    wq = nc.dram_tensor("wq", [128, KT, FPC], F32, kind="ExternalInput")  # /8
    wk = nc.dram_tensor("wk", [128, KT, FPC], F32, kind="ExternalInput")
    wv = nc.dram_tensor("wv", [128, KT, FPC], F32, kind="ExternalInput")
    wo = nc.dram_tensor("wo", [128, 2, D], F32, kind="ExternalInput")
    bq = nc.dram_tensor("bq", [128, 2], F32, kind="ExternalInput")        # /8
    bk = nc.dram_tensor("bk", [128, 2], F32, kind="ExternalInput")
    maskneg = nc.dram_tensor("maskneg", [1, T], F32, kind="ExternalInput")
    mbias = nc.dram_tensor("mbias", [128, TT], F32, kind="ExternalInput")
    onesT = nc.dram_tensor("onesT", [1, T], F32, kind="ExternalInput")
    ones64 = nc.dram_tensor("ones64", [1, DH], F32, kind="ExternalInput")
    onesV = nc.dram_tensor("onesV", [128, TT], F32, kind="ExternalInput")
    att4 = nc.dram_tensor("att4", [HPC, T, T], F32, kind="ExternalOutput")
    yp = nc.dram_tensor("yp", [T, D], F32, kind="ExternalOutput")

    with tile.TileContext(nc) as tc:
        # ---------- persistent SBUF ----------
        with tc.tile_pool(name="persist", bufs=1) as pp:
            # natural-side per-head tiles (f32r, aug row 64)
            qaug = [pp.tile([128, T], F32R, tag=f"qaug{h}", name=f"qaug{h}")
                    for h in range(HPC)]
            kaug = [pp.tile([128, T], F32R, tag=f"kaug{h}", name=f"kaug{h}")
                    for h in range(HPC)]
            # transposed-side pair tiles (bf16; head 2p rows 0-63,
            # 2p+1 rows 64-127)
            qpair = [pp.tile([128, T], BF16, tag=f"qpair{p}", name=f"qpair{p}")
                     for p in range(2)]
            kpair = [pp.tile([128, T], BF16, tag=f"kpair{p}", name=f"kpair{p}")
                     for p in range(2)]
            # V token-major per head (f32r): [tok128, kt, DH + ones col]
            vaug = [pp.tile([128, TT, DH + 1], F32R, tag=f"vaug{h}",
                            name=f"vaug{h}") for h in range(HPC)]
            at01 = pp.tile([128, T], F32R, tag="at01")   # A^T heads 0,1
            at23 = pp.tile([128, T], F32R, tag="at23")   # A^T heads 2,3
            ident = pp.tile([128, 128], F32, tag="ident")
            ones_row = pp.tile([1, DH], F32R, tag="ones_row")
            mb = pp.tile([128, TT], F32, tag="mb")
            wo_sb = pp.tile([128, 2, D], F32R, tag="wo_sb")
            nc.gpsimd.dma_start(out=wo_sb[:], in_=wo[:])

            make_identity(nc, ident[:])
            nc.gpsimd.dma_start(out=ones_row[:], in_=ones64[:])
            nc.sync.dma_start(out=mb[:], in_=mbias[:])
            for h in range(HPC):
                nc.gpsimd.dma_start(out=qaug[h][64:65, :], in_=onesT[:])
                nc.gpsimd.dma_start(out=kaug[h][64:65, :], in_=maskneg[:])
                nc.gpsimd.dma_start(out=vaug[h][:, :, DH:DH + 1],
                                    in_=onesV[:, :, None])

            # ---------- phase 1: projections ----------
            with tc.tile_pool(name="p1w", bufs=1) as p1w, \
                 tc.tile_pool(name="p1x", bufs=2) as p1x, \
                 tc.tile_pool(name="p1v", bufs=4) as p1v, \
                 tc.tile_pool(name="p1ps", bufs=1, space="PSUM") as p1ps:
                w_sb = {}
                for nm, wt in (("q", wq), ("k", wk), ("v", wv)):
                    w_sb[nm] = p1w.tile([128, KT, FPC], F32R, tag=f"w{nm}",
                                        name=f"w{nm}")
                    nc.gpsimd.dma_start(out=w_sb[nm][:], in_=wt[:])
                b_sb = {}
                for nm, bt in (("q", bq), ("k", bk)):
                    b_sb[nm] = p1w.tile([128, 2], F32, tag=f"b{nm}",
                                        name=f"b{nm}")
                    nc.sync.dma_start(out=b_sb[nm][:], in_=bt[:])

                def project(xdram, wname, evac):
                    """PT[f,t] = sum_K W[K,f] X^T[K,t]; evac(ps, m, n)."""
                    ps = [p1ps.tile([128, 512], F32, tag=f"ps{m}_{n}",
                                    name=f"ps{m}_{n}")
                          for m in range(2) for n in range(4)]
                    for kc in range(KT // XC):
                        xt = p1x.tile([128, XC, T], F32R, tag="xstream",
                                      name="xt")
                        nc.gpsimd.dma_start(
                            out=xt[:], in_=xdram[:, kc * XC:(kc + 1) * XC, :])
                        for kj in range(XC):
                            kt = kc * XC + kj
                            for m in range(2):
                                for n in range(4):
                                    nc.tensor.matmul(
                                        ps[m * 4 + n][:],
                                        w_sb[wname][:, kt,
                                                    m * 128:(m + 1) * 128],
                                        xt[:, kj, n * 512:(n + 1) * 512],
                                        start=(kt == 0), stop=(kt == KT - 1))
                    for m in range(2):
                        for n in range(4):
                            evac(ps[m * 4 + n], m, n)

                def evac_qk(dst, pair, bias):
                    def evac(ps, m, n):
                        cols = slice(n * 512, (n + 1) * 512)
                        # per-head f32r rows 0-63 (natural side)
                        for hh in range(2):
                            h = 2 * m + hh
                            nc.vector.tensor_scalar(
                                out=dst[h][0:DH, cols],
                                in0=ps[hh * DH:(hh + 1) * DH, :],
                                scalar1=bias[hh * DH:(hh + 1) * DH, m:m + 1],
                                scalar2=None, op0=ALU.add)
                        # pair bf16 copy (transposed side), no partition shift
                        nc.vector.tensor_scalar(
                            out=pair[m][:, cols], in0=ps[:],
                            scalar1=bias[:, m:m + 1], scalar2=None,
                            op0=ALU.add)
                    return evac

                project(xq, "q", evac_qk(qaug, qpair, b_sb["q"]))
                project(xk, "k", evac_qk(kaug, kpair, b_sb["k"]))

                # V: psum -> block staging -> PE transpose -> vaug
                def evac_v(ps, m, n):
                    for j in range(4):
                        tb = n * 4 + j
                        vb = p1v.tile([128, 128], F32, tag="vblk", name="vblk")
                        nc.vector.tensor_copy(vb[:],
                                              ps[:, j * 128:(j + 1) * 128])
                        pt = p1ps.tile([128, 128], F32, tag=f"ps{m}_{n}",
                                       name="ptr")
                        nc.tensor.transpose(pt[:], vb[:], ident[:])
                        for hh in range(2):
                            h = 2 * m + hh
                            nc.vector.tensor_copy(
                                vaug[h][:, tb, 0:DH],
                                pt[:, hh * DH:(hh + 1) * DH])

                project(xv, "v", evac_v)

            # ---------- phase 2: attention ----------
            with tc.tile_pool(name="p2u", bufs=4) as p2u, \
                 tc.tile_pool(name="p2att", bufs=4) as p2att, \
                 tc.tile_pool(name="p2sm", bufs=3) as p2sm, \
                 tc.tile_pool(name="psAB", bufs=1, space="PSUM") as psAB_p, \
                 tc.tile_pool(name="psV", bufs=2, space="PSUM") as psV_p, \
                 tc.tile_pool(name="psN", bufs=2, space="PSUM") as psN_p:
                for q2 in range(4):      # 512-wide query blocks
                    for p in range(2):
                        hA, hB = 2 * p, 2 * p + 1
                        q0 = q2 * 512
                        qc = slice(q0, q0 + 512)
                        # ---- transposed side: row-packed bf16 pair ----
                        pavA = psV_p.tile([DH + 1, 512], F32, tag="pav",
                                          name="pavA")
                        pavB = psV_p.tile([DH + 1, 512], F32, tag="pav",
                                          name="pavB")
                        for kt in range(TT):
                            kc = slice(kt * 128, (kt + 1) * 128)
                            psab = psAB_p.tile([128, 1024], F32, tag="psab",
                                               name="psab")
                            nc.tensor.matmul(psab[:, 0:512],
                                             kpair[p][0:DH, kc],
                                             qpair[p][0:DH, qc],
                                             start=True, stop=True)
                            nc.tensor.matmul(psab[:, 512:1024],
                                             kpair[p][DH:128, kc],
                                             qpair[p][DH:128, qc],
                                             start=True, stop=True,
                                             tile_position=(64, 0))
                            uA = p2u.tile([128, 512], F32R, tag="uA",
                                          name="uA")
                            nc.scalar.activation(uA[:], psab[:, 0:512], AF.Exp,
                                                 bias=mb[:, kt:kt + 1])
                            uB = p2u.tile([128, 512], F32R, tag="uB",
                                          name="uB")
                            nc.scalar.activation(uB[:], psab[:, 512:1024],
                                                 AF.Exp,
                                                 bias=mb[:, kt:kt + 1])
                            nc.tensor.matmul(pavA[:], vaug[hA][:, kt, :],
                                             uA[:], start=(kt == 0),
                                             stop=(kt == TT - 1))
                            nc.tensor.matmul(pavB[:], vaug[hB][:, kt, :],
                                             uB[:], start=(kt == 0),
                                             stop=(kt == TT - 1))

                        # ---- natural side + denominators, heads A and B ----
                        for h in (hA, hB):
                            for qs in range(4):
                                row0 = q0 + qs * 128
                                asb = p2att.tile([128, T], F32, tag="asb",
                                                 name="asb")
                                accs = p2sm.tile([128, 2], F32, tag="accs",
                                                 name="accs")
                                for kn in range(2):
                                    psn = psN_p.tile([128, 1024], F32,
                                                     tag="psn", name="psn")
                                    for jj in range(2):
                                        nc.tensor.matmul(
                                            psn[:, jj * 512:(jj + 1) * 512],
                                            qaug[h][0:DH + 1, row0:row0 + 128],
                                            kaug[h][0:DH + 1,
                                                    (2 * kn + jj) * 512:
                                                    (2 * kn + jj + 1) * 512],
                                            start=True, stop=True)
                                    nc.scalar.activation(
                                        asb[:, kn * 1024:(kn + 1) * 1024],
                                        psn[:], AF.Exp,
                                        accum_out=accs[:, kn:kn + 1])
                                # denom = sum of chunk accums; reciprocal
                                dcol = p2sm.tile([128, 1], F32, tag="dcol",
                                                 name="dcol")
                                nc.vector.reduce_sum(dcol[:], accs[:],
                                                     axis=AX.X)
                                rcol = p2sm.tile([128, 1], F32, tag="rcol",
                                                 name="rcol")
                                nc.vector.reciprocal(rcol[:], dcol[:])
                                # normalize att rows in place, then DMA out
                                nc.vector.tensor_scalar(
                                    out=asb[:], in0=asb[:], scalar1=rcol[:],
                                    scalar2=None, op0=ALU.mult)
                                nc.sync.dma_start(
                                    out=att4[h, row0:row0 + 128, :],
                                    in_=asb[:])

                        # ---- outT normalize into AT tiles (denoms = AV row
                        # 64; 1/d = exp(-ln d) on ACT) ----
                        at_dst = at01 if p == 0 else at23
                        for h, pav in ((hA, pavA), (hB, pavB)):
                            lnr = p2sm.tile([1, 512], F32, tag="lnr",
                                            name="lnr")
                            nc.scalar.activation(lnr[:], pav[DH:DH + 1, :],
                                                 AF.Ln)
                            rrw = p2sm.tile([1, 512], F32R, tag="rrw",
                                            name="rrw")
                            nc.scalar.activation(rrw[:], lnr[:], AF.Exp,
                                                 scale=-1.0)
                            prc = psN_p.tile([DH, 512], F32, tag="psn",
                                             name="prc")
                            nc.tensor.matmul(prc[:], ones_row[:],
                                             rrw[:], start=True,
                                             stop=True)
                            rbc = p2sm.tile([DH, 512], F32, tag="rbc",
                                            name="rbc")
                            nc.vector.tensor_copy(rbc[:], prc[:])
                            nc.vector.tensor_mul(
                                at_dst[(h % 2) * DH:(h % 2) * DH + DH, qc],
                                pav[0:DH, :], rbc[:])

                    # ---- output projection for this q2 block ----
                    for tj in range(4):
                        ts_ = q2 * 4 + tj
                        ysb = p2att.tile([128, D], F32, tag="ysb", name="ysb")
                        for n in range(2):
                            py = psAB_p.tile([128, 512], F32, tag="psab",
                                             name="py")
                            nc.tensor.matmul(
                                py[:], at01[:, ts_ * 128:(ts_ + 1) * 128],
                                wo_sb[:, 0, n * 512:(n + 1) * 512],
                                start=True, stop=False)
                            nc.tensor.matmul(
                                py[:], at23[:, ts_ * 128:(ts_ + 1) * 128],
                                wo_sb[:, 1, n * 512:(n + 1) * 512],
                                start=False, stop=True)
                            nc.vector.tensor_copy(
                                ysb[:, n * 512:(n + 1) * 512], py[:])
                        nc.gpsimd.dma_start(
                            out=yp[ts_ * 128:(ts_ + 1) * 128, :], in_=ysb[:])

    nc.compile()
    return nc


_NC = None


def kernel(query=None, key_in=None, value=None, mask=None, Wq=None, bq=None,
           Wk=None, bk=None, Wv=None, bv=None, Wo=None, bo=None, key=None,
           **_ignored):
    global _NC, LAST_RESULTS
    if key_in is None:
        key_in = key
    query = np.ascontiguousarray(np.asarray(query, dtype=np.float32))
    key_in = np.ascontiguousarray(np.asarray(key_in, dtype=np.float32))
    value = np.ascontiguousarray(np.asarray(value, dtype=np.float32))
    mask_np = np.asarray(mask)
    Wq = np.asarray(Wq, dtype=np.float32)
    Wk = np.asarray(Wk, dtype=np.float32)
    Wv = np.asarray(Wv, dtype=np.float32)
    Wo = np.asarray(Wo, dtype=np.float32)
    bq_np = np.asarray(bq, dtype=np.float32)
    bk_np = np.asarray(bk, dtype=np.float32)
    bv_np = np.asarray(bv, dtype=np.float32)
    bo_np = np.asarray(bo, dtype=np.float32)

    if _NC is None:
        _NC = build_nc()

    scale = 1.0 / np.sqrt(np.float32(DH))
    xT = {}
    for b in range(B):
        # [128, KT, T]: partition-major X^T
        xT[("q", b)] = np.ascontiguousarray(
            query[b].T.reshape(KT, 128, T).transpose(1, 0, 2))
        xT[("k", b)] = np.ascontiguousarray(
            key_in[b].T.reshape(KT, 128, T).transpose(1, 0, 2))
        xT[("v", b)] = np.ascontiguousarray(
            value[b].T.reshape(KT, 128, T).transpose(1, 0, 2))
    maskneg = {b: (mask_np[b, 0, 0, :].astype(np.float32) * np.float32(-1e9))
               for b in range(B)}

    in_maps = []
    for c in range(NCORES):
        b, g = c // HPC, c % HPC
        F = slice(g * FPC, (g + 1) * FPC)
        in_maps.append({
            "xq": xT[("q", b)],
            "xk": xT[("k", b)],
            "xv": xT[("v", b)],
            "wq": np.ascontiguousarray(
                (Wq[:, F] * scale).reshape(KT, 128, FPC).transpose(1, 0, 2)),
            "wk": np.ascontiguousarray(
                Wk[:, F].reshape(KT, 128, FPC).transpose(1, 0, 2)),
            "wv": np.ascontiguousarray(
                Wv[:, F].reshape(KT, 128, FPC).transpose(1, 0, 2)),
            "wo": np.ascontiguousarray(
                Wo[F, :].reshape(2, 128, D).transpose(1, 0, 2)),
            "bq": np.ascontiguousarray((bq_np[F] * scale).reshape(2, 128).T),
            "bk": np.ascontiguousarray(bk_np[F].reshape(2, 128).T),
            "maskneg": maskneg[b].reshape(1, T),
            "mbias": np.ascontiguousarray(maskneg[b].reshape(TT, 128).T),
            "onesT": np.ones((1, T), np.float32),
            "ones64": np.ones((1, DH), np.float32),
            "onesV": np.ones((128, TT), np.float32),
        })

    res = run_bass_kernel_spmd(_NC, in_maps, list(range(NCORES)), trace=TRACE)
    LAST_RESULTS = res

    att_w = np.empty((B, H, T, T), dtype=np.float32)
    out = np.zeros((B, T, D), dtype=np.float32)
    for c in range(NCORES):
        b, g = c // HPC, c % HPC
        att_w[b, g * HPC:(g + 1) * HPC] = res.results[c]["att4"]
        out[b] += res.results[c]["yp"]
    out += (bv_np @ Wo + bo_np)[None, None, :]
    return out, att_w
